# revision 1
# baseline (speedup 1.0000x reference)
"""FAGCN message-passing layer on 8 Trainium2 NeuronCores (Bass/Tile).

Strategy (v3: dst-sharded, degree-sorted dst-per-partition windows,
bulk dma_gather over an int16-range-split node table):
  - Nodes are 1D-partitioned across 8 cores by dst (12544/core). Each
    core's dst nodes are sorted by in-degree and packed into 98 windows
    of 128; window w partition p owns one dst node, so slot counts per
    partition track the window's (near-uniform) degree.
  - Per-core node TABLE: Haug[row] = [h*d (64 f16), gs, gs, pad] in
    256B rows (dma_gather granularity), with rows ordered per core and
    split into 4 ranges of <=32768 rows so gather indices fit int16.
    A greedy balanced coloring assigns each referenced src node to a
    range so that every dst node's edges split evenly across ranges
    (minimizes per-window slot padding). All table compute (h*d, h@W)
    runs on device; the host only chooses row order / indices.
  - Main loop: for each pair of windows, one dma_gather per range
    fetches all needed src rows (int16 indices, 256B elems); ACT
    computes th=tanh(gs+gd_dst) with gd as per-partition bias; DVE
    multiplies messages into the gathered tile's pad columns and
    reduces over the slot axis; z row = d_dst * sum. z is written
    partition-major and un-permuted on the host.
"""
import numpy as np

P = 128
D = 64
EL = 128          # table row: h' (64) + gs (65) + gs dup + pad = 256B
N_CORES = 8
NPC = 12544
NW = NPC // P     # 98
N_NODES_MAX = 100352
R2 = 102400       # padded table rows (50 * 2048)
NRANGE = 4
RSTART = [0, 32768, 65536, 98304]
RCAP = [32767, 32767, 32767, 4095]   # last row of each range = zero row
ZROWR = [32767, 32767, 32767, 4095]  # in-range index of the zero row
HL_ROWS = 13312   # permuted local-h rows padded to 104 windows
BB_A = 16         # build-A nodes per tile-row (50 iters of 2048 rows)
GRP = 16          # windows per gather group
GZ = 14           # windows per z flush
HC = 66           # gathered row content: h' (64) + gs (1) + gs dup


def _color_ranges(src_e, dl_e, npc):
    """Greedy balanced range coloring: assign each referenced src node a
    range 0..2 (overflow 3) minimizing per-dst edge imbalance."""
    order_e = np.argsort(src_e, kind="stable")
    ss = src_e[order_e]
    dd = dl_e[order_e]
    uniq, starts = np.unique(ss, return_index=True)
    ends = np.append(starts[1:], ss.size)
    refcnt = ends - starts
    # process srcs by refcount desc (high-impact first)
    proc = np.argsort(-refcnt, kind="stable")
    color = np.full(N_NODES_MAX, 3, np.int8)
    cnt = np.zeros((npc, 3), np.int32)
    fill = [0, 0, 0]
    for k in proc:
        s = uniq[k]
        dsts = dd[starts[k]:ends[k]]
        score = cnt[dsts, :].sum(axis=0)
        for g in np.argsort(score, kind="stable"):
            if fill[g] < RCAP[g]:
                break
        else:
            g = 3
        color[s] = g
        if g < 3:
            fill[g] += 1
            np.add.at(cnt, (dsts, g), 1)
    return color, uniq, refcnt


def _idx_layout(NTWG):
    """Pass-major idx column offsets: blocks ordered (g, group, w) so each
    (pass, group) gather's indices are one contiguous block."""
    icol_off = np.zeros((NW, NRANGE), np.int64)
    c = 0
    for g in range(NRANGE):
        for w0 in range(0, NW, GRP):
            for w in range(w0, min(w0 + GRP, NW)):
                icol_off[w, g] = c
                c += int(NTWG[w, g]) * 8
    return icol_off, c


def _host_prep(h, d, gate_W, gate_b, edge_src, edge_dst):
    """Shard + layout preparation (pure data movement / indexing)."""
    N = h.shape[0]
    h_pad = np.zeros((R2, D), dtype=np.float32)
    h_pad[:N] = np.asarray(h, dtype=np.float32)
    d_pad = np.zeros((R2,), dtype=np.float32)
    d_pad[:N] = np.asarray(d, dtype=np.float32)

    WSRC = np.tile(np.asarray(gate_W[0, D:2 * D], np.float32), (P, 1))
    WDST = np.tile(np.asarray(gate_W[0, 0:D], np.float32), (P, 1))
    BREP = np.full((P, 1), float(np.asarray(gate_b).reshape(-1)[0]), np.float32)

    order = np.argsort(edge_dst, kind="stable")
    sd = np.asarray(edge_dst)[order].astype(np.int64)
    ss = np.asarray(edge_src)[order].astype(np.int64)
    bounds = np.searchsorted(sd, np.arange(N_CORES + 1) * NPC)

    cores = []
    for c in range(N_CORES):
        lo, hi = int(bounds[c]), int(bounds[c + 1])
        dl = sd[lo:hi] - c * NPC
        src = ss[lo:hi]

        color, uniq, refcnt = _color_ranges(src, dl, NPC)

        # table row assignment: per range, referenced srcs by refcount desc
        tau = np.full(N_NODES_MAX, -1, np.int64)
        g_all = np.full(N_NODES_MAX, -1, np.int8)
        g_all[uniq] = color[uniq]
        used = np.zeros(NRANGE, np.int64)
        rc_full = np.zeros(N_NODES_MAX, np.int64)
        rc_full[uniq] = refcnt
        for g in range(NRANGE):
            nodes_g = uniq[color[uniq] == g]
            nodes_g = nodes_g[np.argsort(-rc_full[nodes_g], kind="stable")]
            assert nodes_g.size <= RCAP[g], (g, nodes_g.size)
            tau[nodes_g] = RSTART[g] + np.arange(nodes_g.size)
            used[g] = nodes_g.size
        # unreferenced nodes: stuff anywhere with free rows (never gathered)
        unref = np.where(g_all < 0)[0]
        pos = 0
        for g in range(NRANGE):
            free = RCAP[g] - used[g]
            take = min(free, unref.size - pos)
            if take > 0:
                tau[unref[pos:pos + take]] = RSTART[g] + used[g] + np.arange(take)
                used[g] += take
                pos += take
        assert pos == unref.size

        # per-edge range + rank within (dst, range)
        ge = g_all[src].astype(np.int64)
        key = dl * NRANGE + ge
        c_g = np.bincount(key, minlength=NPC * NRANGE).reshape(NPC, NRANGE)
        deg = c_g.sum(axis=1)
        kstart = np.zeros(NPC * NRANGE, np.int64)
        kstart[1:] = np.cumsum(np.bincount(key, minlength=NPC * NRANGE))[:-1]
        ord2 = np.argsort(key, kind="stable")
        trank = np.empty(src.size, np.int64)
        trank[ord2] = np.arange(src.size) - kstart[key[ord2]]

        # window packing: group nodes by their worst per-range count so the
        # per-window max (= slot padding) tracks the mean
        perm = np.argsort(-(c_g[:, :3].max(axis=1) * 64 + deg), kind="stable")
        rankof = np.empty(NPC, np.int64)
        rankof[perm] = np.arange(NPC)

        ntwg_c = c_g[perm].reshape(NW, P, NRANGE).max(axis=1)  # [NW, 4]
        cores.append(dict(
            dl=dl, src=src, tau=tau, ge=ge, trank=trank, perm=perm,
            rankof=rankof, ntwg=ntwg_c,
        ))

    NTWG = np.maximum.reduce([cc["ntwg"] for cc in cores])     # [NW, 4]
    key = tuple(int(x) for x in NTWG.reshape(-1))
    icol_off, TOTI = _idx_layout(NTWG)

    in_maps = []
    for c in range(N_CORES):
        cc = cores[c]
        tau, ge, trank, perm, rankof = (cc["tau"], cc["ge"], cc["trank"],
                                        cc["perm"], cc["rankof"])
        dl, src = cc["dl"], cc["src"]

        r = rankof[dl]
        p_arr = r % P
        w_arr = r // P

        # flat slot index within (w,g): i = trank*128 + p
        idxw = np.zeros((16, TOTI), np.int16)
        # initialize pads to the per-range zero row
        for w in range(NW):
            for g in range(NRANGE):
                nt = int(NTWG[w, g])
                if nt == 0:
                    continue
                o = int(icol_off[w, g])
                idxw[:, o:o + nt * 8] = ZROWR[g]
        sel_i = trank * P + p_arr                     # position within block
        col16 = sel_i // 16
        row16 = sel_i % 16
        val = (tau[src] - np.asarray(RSTART, np.int64)[ge]).astype(np.int64)
        dest_col = icol_off[w_arr, ge] + col16
        idxw[row16, dest_col] = val.astype(np.int16)
        IDXW = np.tile(idxw, (8, 1))                  # replicate per 16-group

        # permuted local h for gd (2-row-packed build-B layout)
        perm_pad = np.full(HL_ROWS, c * NPC, np.int64)
        perm_pad[:NPC] = perm + c * NPC
        rr = np.arange(HL_ROWS)
        jj, rem = rr // 256, rr % 256
        pp_, two = rem // 2, rem % 2
        hloc2 = h_pad[perm_pad[(2 * jj + two) * P + pp_]]

        dcol = d_pad[perm + c * NPC].reshape(NW, P).T.copy()

        # per-core table-ordered h and d (4-row-packed build-A layout)
        node_of_row = np.full(R2, -1, np.int64)
        node_of_row[tau[tau >= 0]] = np.where(tau >= 0)[0]
        h_tab = np.zeros((R2, D), np.float32)
        d_tab = np.zeros((R2,), np.float32)
        m = node_of_row >= 0
        h_tab[m] = h_pad[node_of_row[m]]
        d_tab[m] = d_pad[node_of_row[m]]
        DROWS4 = d_tab.reshape(R2 // 512, 128, 4).transpose(1, 0, 2).reshape(P, R2 // P)
        DROWS4 = np.ascontiguousarray(DROWS4)

        in_maps.append({
            "h_tab": h_tab, "drows4": DROWS4, "wsrc": WSRC, "wdst": WDST,
            "brep": BREP, "idxw": np.ascontiguousarray(IDXW),
            "hloc2": hloc2, "dcol": dcol,
            "_perm": perm,
        })
    return in_maps, key


def _dma_gather_raw(eng, mybir, out_ap, in_ap, idxs_ap, num_idxs,
                    elem_size, elem_step):
    """dma_gather with elem_size < row stride (content-only fetch).  The ISA
    encodes the row stride in 256B units but elem_size is a free byte count;
    bass's helper asserts elem_size%256==0, so construct the instruction
    directly (semantics verified on hardware)."""
    stride_bytes = elem_step * mybir.dt.size(in_ap.dtype)
    assert stride_bytes % 256 == 0
    _in_ap = eng.lower_ap_dma(in_ap, for_custom_bir_dma=True)
    _idxs_ap = eng.lower_ap(idxs_ap)
    _out_ap = eng.lower_ap(out_ap)
    return eng.add_instruction(
        mybir.InstDMAGatherAnt(
            name=eng.bass.get_next_instruction_name(),
            ins=[*_in_ap, _idxs_ap,
                 eng.lower_val_access(eng.to_reg(num_idxs))],
            outs=[_out_ap],
            transpose=False,
            num_idxs=num_idxs,
            elem_size=elem_size,
            stride_bytes_256=stride_bytes // 256,
            gen_mode=0,
            single_packet=True,
            queue_num=0,
            sbuf_tokens_per_rank=0,
            sbuf_free_dim_per_rank=0,
            sbuf_free_dim_pad_per_rank=0,
            sbuf_byte_offset=0,
        ))


def _build_program(key):
    import concourse.bacc as bacc
    import concourse.tile as tile
    from concourse import bass, mybir

    NTWG = np.asarray(key, np.int64).reshape(NW, NRANGE)
    icol_off, TOTI = _idx_layout(NTWG)
    grps = [list(range(w0, min(w0 + GRP, NW))) for w0 in range(0, NW, GRP)]
    S2MAX = int(max(sum(int(NTWG[w, g]) for w in ws)
                    for ws in grps for g in range(NRANGE)))
    S2MAX = max(S2MAX, 1)

    f32, f16 = mybir.dt.float32, mybir.dt.float16
    i16 = mybir.dt.int16

    nc = bacc.Bacc("TRN2", target_bir_lowering=False, debug=False,
                   num_devices=N_CORES)
    htab_d = nc.dram_tensor("h_tab", [R2, D], f32, kind="ExternalInput")
    drows4_d = nc.dram_tensor("drows4", [P, R2 // P], f32, kind="ExternalInput")
    wsrc_d = nc.dram_tensor("wsrc", [P, D], f32, kind="ExternalInput")
    wdst_d = nc.dram_tensor("wdst", [P, D], f32, kind="ExternalInput")
    brep_d = nc.dram_tensor("brep", [P, 1], f32, kind="ExternalInput")
    idxw_d = nc.dram_tensor("idxw", [P, TOTI], i16, kind="ExternalInput")
    hloc2_d = nc.dram_tensor("hloc2", [HL_ROWS, D], f32, kind="ExternalInput")
    dcol_d = nc.dram_tensor("dcol", [P, NW], f32, kind="ExternalInput")
    z_d = nc.dram_tensor("z", [P, NW, D], f32, kind="ExternalOutput")

    # one table tensor per range so pass-g gathers depend only on the
    # range-g build (build overlaps the Pool-bound main loop)
    haug_g = [nc.dram_tensor(f"haug_g{g}", [RCAP[g] + 1, EL], f16,
                             kind="Internal") for g in range(NRANGE)]

    with tile.TileContext(nc) as tc:
        with tc.tile_pool(name="const", bufs=1) as cp:
            dcol_t = cp.tile([P, NW], f32)
            nc.sync.dma_start(out=dcol_t[:], in_=dcol_d[:, :])
            gdw_t = cp.tile([P, HL_ROWS // P], f32)

            # build + main pools stay open together so the scheduler can
            # overlap the build with the first gather passes
            with (tc.tile_pool(name="bld", bufs=2) as bp,
                  tc.tile_pool(name="main", bufs=2) as mp):
                wsrc_t = cp.tile([P, D], f32)
                nc.sync.dma_start(out=wsrc_t[:], in_=wsrc_d[:, :])
                wdst_t = cp.tile([P, D], f32)
                nc.sync.dma_start(out=wdst_t[:], in_=wdst_d[:, :])
                brep_t = cp.tile([P, 1], f32)
                nc.sync.dma_start(out=brep_t[:], in_=brep_d[:, :])
                drows4_t = cp.tile([P, R2 // P], f32)
                nc.sync.dma_start(out=drows4_t[:], in_=drows4_d[:, :])

                # build A: table rows [h*d (64), gs, gs] from h_tab.
                # h*d runs on the (otherwise idle) ACT engine as per-column
                # Copy activations with per-partition scale; the gs dot
                # (mult+reduce) stays on DVE.
                def emit_build_chunk(g, s):
                    rows = slice(s * BB_A * P, (s + 1) * BB_A * P)
                    grows = slice(RSTART[g] + s * BB_A * P,
                                  RSTART[g] + (s + 1) * BB_A * P)
                    h8 = bp.tile([P, 4, 4 * D], f32, tag="h8")
                    nc.sync.dma_start(
                        out=h8[:],
                        in_=htab_d[grows, :].rearrange(
                            "(j p four) e -> p j (four e)", p=P, four=4))
                    h8v = h8[:].rearrange("p j (f e) -> p j f e", e=D)
                    hp16 = bp.tile([P, 4, 4, EL], f16, tag="hp16")
                    dc0 = (RSTART[g] // 512 + s * 4) * 4
                    for jf in range(BB_A):
                        j, f = jf // 4, jf % 4
                        nc.scalar.activation(
                            out=hp16[:, j, f, 0:D], in_=h8v[:, j, f, :],
                            func=mybir.ActivationFunctionType.Copy,
                            scale=drows4_t[:, dc0 + jf:dc0 + jf + 1])
                    prod = bp.tile([P, 4, 4, D], f32, tag="prod")
                    nc.vector.tensor_tensor(
                        out=prod[:], in0=h8v,
                        in1=wsrc_t[:].rearrange(
                            "p (a b e) -> p a b e", a=1, b=1
                        ).to_broadcast([P, 4, 4, D]),
                        op=mybir.AluOpType.mult)
                    gsb = bp.tile([P, 4, 4], f32, tag="gsb")
                    nc.vector.tensor_reduce(out=gsb[:], in_=prod[:],
                                            op=mybir.AluOpType.add,
                                            axis=mybir.AxisListType.X)
                    # gs (+bias) broadcast over cols 64:128 (fills row pad so
                    # the 512B-chunk DMA below reads fully-written SBUF)
                    nc.vector.tensor_scalar(
                        out=hp16[:, :, :, D:EL],
                        in0=gsb[:].rearrange("p j (f a) -> p j f a",
                                             a=1).to_broadcast([P, 4, 4, EL - D]),
                        scalar1=brep_t[:, 0:1], scalar2=None,
                        op0=mybir.AluOpType.add)
                    nc.sync.dma_start(
                        out=haug_g[g][rows, :].rearrange(
                            "(j p four) e -> p j (four e)", p=P, four=4),
                        in_=hp16[:].rearrange("p j f e -> p j (f e)"))

                # emit range 0 up front; later ranges interleave with passes
                nchunk = [(RCAP[g] + 1) // (BB_A * P) for g in range(NRANGE)]
                for s in range(nchunk[0]):
                    emit_build_chunk(0, s)

                # build B: gd for local (window-permuted) nodes
                for s in range(HL_ROWS // (8 * P)):
                    rows = slice(s * 8 * P, (s + 1) * 8 * P)
                    hl8 = bp.tile([P, 4, 2 * D], f32, tag="h8")
                    nc.sync.dma_start(
                        out=hl8[:],
                        in_=hloc2_d[rows, :].rearrange(
                            "(j p two) e -> p j (two e)", p=P, two=2))
                    prodb = bp.tile([P, 4, 2, D], f32, tag="prodb")
                    nc.vector.tensor_tensor(
                        out=prodb[:],
                        in0=hl8[:].rearrange("p j (t e) -> p j t e", e=D),
                        in1=wdst_t[:].rearrange(
                            "p (a b e) -> p a b e", a=1, b=1
                        ).to_broadcast([P, 4, 2, D]),
                        op=mybir.AluOpType.mult)
                    nc.vector.tensor_reduce(
                        out=gdw_t[:, s * 8:(s + 1) * 8].rearrange(
                            "p (j two) -> p j two", two=2),
                        in_=prodb[:], op=mybir.AluOpType.add,
                        axis=mybir.AxisListType.X)

                # ---------- main: range-major passes over window groups ----
                # z accumulates in SBUF across passes; pass g only waits for
                # the range-g table build.
                zbuf = cp.tile([P, NW, D], f32)
                nc.vector.memset(zbuf[:], 0.0)

                def _flush_group(ws):
                    # scale this group's windows by d_dst and write z out
                    w0, n = ws[0], len(ws)
                    zo = mp.tile([P, GRP, D], f32, tag="zo")
                    nc.vector.tensor_tensor(
                        out=zo[:, 0:n, :], in0=zbuf[:, w0:w0 + n, :],
                        in1=dcol_t[:, w0:w0 + n].rearrange(
                            "p (w a) -> p w a", a=1).to_broadcast([P, n, D]),
                        op=mybir.AluOpType.mult)
                    nc.sync.dma_start(out=z_d[:, w0:w0 + n, :],
                                      in_=zo[:, 0:n, :])

                for g in range(NRANGE):
                    # interleave the next range's build chunks with this
                    # pass's groups so the scheduler overlaps them
                    if g + 1 < NRANGE:
                        pend = list(range(nchunk[g + 1]))
                    else:
                        pend = []
                    # front-load the next range's build into the first ~60%
                    # of this pass's groups so its gathers start unstalled
                    nfront = max(1, (len(grps) * 3) // 5)
                    per = -(-len(pend) // nfront) if pend else 0
                    for gi, ws in enumerate(grps):
                        for _ in range(per):
                            if pend:
                                emit_build_chunk(g + 1, pend.pop(0))
                        S2 = sum(int(NTWG[w, g]) for w in ws)
                        if S2 == 0:
                            if g == NRANGE - 1:
                                _flush_group(ws)
                            continue
                        toff = {}
                        t = 0
                        for w in ws:
                            toff[w] = t
                            t += int(NTWG[w, g])
                        blk_base = int(icol_off[ws[0], g])
                        ga = mp.tile([P, S2MAX, HC], f16, tag="ga")
                        th2 = mp.tile([P, S2MAX, 2], f16, tag="th2")
                        idx_t = mp.tile([P, S2MAX * 8], i16, tag="idx")
                        nc.sync.dma_start(
                            out=idx_t[:, 0:S2 * 8],
                            in_=idxw_d[:, blk_base:blk_base + S2 * 8])
                        # chunked gathers (<=1024 indices per op)
                        for co in range(0, S2, 8):
                            nt = min(8, S2 - co)
                            _dma_gather_raw(
                                nc.gpsimd, mybir,
                                ga[:, co:co + nt, :],
                                haug_g[g][:, :],
                                idx_t[:, co * 8:(co + nt) * 8],
                                nt * P, HC, EL)
                        for w in ws:
                            nt = int(NTWG[w, g])
                            if nt == 0:
                                continue
                            o = toff[w]
                            # th duplicated into [*, 2] (two strided ACT
                            # writes) so the msg multiply's broadcast operand
                            # has a packed 2-elem last dim -> DVE 2x mode
                            for half in range(2):
                                nc.scalar.activation(
                                    out=th2[:, o:o + nt, half],
                                    in_=ga[:, o:o + nt, D],
                                    func=mybir.ActivationFunctionType.Tanh,
                                    bias=gdw_t[:, w:w + 1])
                            nc.vector.tensor_tensor(
                                out=ga[:, o:o + nt, 0:D].rearrange(
                                    "p t (c two) -> p t c two", two=2),
                                in0=ga[:, o:o + nt, 0:D].rearrange(
                                    "p t (c two) -> p t c two", two=2),
                                in1=th2[:, o:o + nt, :].rearrange(
                                    "p t (a two) -> p t a two", a=1
                                ).to_broadcast([P, nt, D // 2, 2]),
                                op=mybir.AluOpType.mult)
                            if g == 0:
                                nc.vector.tensor_reduce(
                                    out=zbuf[:, w, :],
                                    in_=ga[:, o:o + nt, 0:D].rearrange(
                                        "p t d -> p d t"),
                                    op=mybir.AluOpType.add,
                                    axis=mybir.AxisListType.X)
                            else:
                                ztmp = mp.tile([P, D], f32, tag="ztmp")
                                nc.vector.tensor_reduce(
                                    out=ztmp[:],
                                    in_=ga[:, o:o + nt, 0:D].rearrange(
                                        "p t d -> p d t"),
                                    op=mybir.AluOpType.add,
                                    axis=mybir.AxisListType.X)
                                nc.vector.tensor_tensor(
                                    out=zbuf[:, w, :], in0=zbuf[:, w, :],
                                    in1=ztmp[:], op=mybir.AluOpType.add)
                        if g == NRANGE - 1:
                            _flush_group(ws)

    nc.compile()
    return nc


_CACHE = {}


def kernel(h, d, gate_W, gate_b, edge_src, edge_dst):
    from concourse.bass_utils import run_bass_kernel_spmd

    N = h.shape[0]
    in_maps, key = _host_prep(h, d, gate_W, gate_b, edge_src, edge_dst)
    if key not in _CACHE:
        _CACHE[key] = _build_program(key)
    nc = _CACHE[key]
    perms = [m.pop("_perm") for m in in_maps]
    res = run_bass_kernel_spmd(nc, in_maps, core_ids=list(range(N_CORES)))
    z = np.empty((N_CORES * NPC, D), np.float32)
    for c in range(N_CORES):
        zc = res.results[c]["z"]                       # [128, NW, 64]
        zperm = zc.transpose(1, 0, 2).reshape(NPC, D)  # rank-major
        z[perms[c] + c * NPC] = zperm
    return np.ascontiguousarray(z[:N]).astype(np.float32)



# revision 7
# speedup vs baseline: 1.8374x; 1.8374x over previous
"""FAGCN message-passing layer on 8 Trainium2 NeuronCores (Bass/Tile).

Strategy (v4: int8 table rows + PE identity-matmul segment reduction):
  - Nodes 1D-partitioned across 8 cores by dst (12544/core), degree-
    sorted into 98 windows of 128; window w partition p owns one dst.
  - Per-core node TABLE (4 int16-range tensors, 256B rows, host-staged):
    row = [q int8[64] (h per-row-max-quantized), scale*d_src f16, gs f16].
    gs = h @ W_src is computed ON DEVICE (DVE mult + tree reduce over a
    dense f16 copy of the table rows) and written into the 2-byte column.
  - Main loop: one raw dma_gather per (window-group, range) fetches 68B
    rows (the 7ns/desc DMA floor) at ~0.40-0.44 ns/edge; per-window DVE
    gd-add + one batched ACT tanh; th2 pair tile; ACT bulk int8->f16
    dequant; one DVE 2x multiply applies tanh*scale per slot.
  - Reduction on the TENSOR engine: per slot-column matmul with a
    constant 128x128 identity as stationary accumulates messages into
    per-window PSUM tiles (start/stop once per 2KB bank); ACT evacuates
    PSUM with the d_dst scale fused. z un-permuted on the host.
"""
import numpy as np

P = 128
D = 64
EL = 256          # table row stride bytes
CB = 68           # gathered content bytes: 64 q + 2 scale' + 2 gs
N_CORES = 8
NPC = 12544
NW = NPC // P     # 98
N_NODES_MAX = 100352
R2 = 102400       # total table rows
NRANGE = 4
RSTART = [0, 32768, 65536, 98304]
RCAP = [32767, 32767, 32767, 4095]   # last row of each range = zero row
ZROWR = [32767, 32767, 32767, 4095]  # in-range index of the zero row
GRP = 16          # windows per gather group
GRPS = [list(range(w0, min(w0 + GRP, NW))) for w0 in range(0, NW, GRP)]
# PSUM sections: 32 windows = 4 banks each; ring of 2 tiles = 8 banks
SECS = [GRPS[0:2], GRPS[2:4], GRPS[4:6], GRPS[6:]]
SW0 = [0, 32, 64, 96]                # first window of each section
SNW = [32, 32, 32, 2]                # windows per section
GS_CHUNK = 64                        # hdense j-columns per gs chunk


def _color_ranges(src_e, dl_e, npc):
    """Greedy balanced range coloring: assign each referenced src node a
    range 0..2 (overflow 3) minimizing per-dst edge imbalance."""
    order_e = np.argsort(src_e, kind="stable")
    ss = src_e[order_e]
    dd = dl_e[order_e]
    uniq, starts = np.unique(ss, return_index=True)
    ends = np.append(starts[1:], ss.size)
    refcnt = ends - starts
    proc = np.argsort(-refcnt, kind="stable")
    color = np.full(N_NODES_MAX, 3, np.int8)
    cnt = np.zeros((npc, 3), np.int32)
    fill = [0, 0, 0]
    for k in proc:
        s = uniq[k]
        dsts = dd[starts[k]:ends[k]]
        score = cnt[dsts, :].sum(axis=0)
        for g in np.argsort(score, kind="stable"):
            if fill[g] < RCAP[g]:
                break
        else:
            g = 3
        color[s] = g
        if g < 3:
            fill[g] += 1
            np.add.at(cnt, (dsts, g), 1)
    return color, uniq, refcnt


def _idx_layout(NTWG):
    """Idx column offsets: blocks ordered (half, g, grp, w) so each
    (half, g, grp) gather's indices are one contiguous block."""
    icol_off = np.zeros((NW, NRANGE), np.int64)
    c = 0
    for sec in SECS:
        for g in range(NRANGE):
            for ws in sec:
                for w in ws:
                    icol_off[w, g] = c
                    c += int(NTWG[w, g]) * 8
    return icol_off, c


def _mm_stream(NTWG):
    """Per-bank first/last matmul (g, w, t) in emission order."""
    first = {}
    last = {}
    for sec in SECS:
        for g in range(NRANGE):
            for ws in sec:
                for w in ws:
                    for t in range(int(NTWG[w, g])):
                        b = w // 8
                        if b not in first:
                            first[b] = (g, w, t)
                        last[b] = (g, w, t)
    return first, last


def _host_prep(h, d, gate_W, gate_b, edge_src, edge_dst):
    """Shard + layout preparation (data movement, permutation, int8
    transport quantization). All gate-projection FLOPs stay on device."""
    N = h.shape[0]
    h32 = np.asarray(h, dtype=np.float32)
    h_pad = np.zeros((N_NODES_MAX, D), dtype=np.float32)
    h_pad[:N] = h32
    d_pad = np.zeros((N_NODES_MAX,), dtype=np.float32)
    d_pad[:N] = np.asarray(d, dtype=np.float32)

    # int8 transport quantization of h (per-row scale)
    scale = np.abs(h_pad).max(axis=1) / 127.0
    safe = np.where(scale > 0, scale, 1.0)
    q_all = np.clip(np.round(h_pad / safe[:, None]), -127, 127).astype(np.int8)
    h16_all = h_pad.astype(np.float16)

    WSRC = np.tile(np.asarray(gate_W[0, D:2 * D], np.float16), (P, 1))
    WDST = np.tile(np.asarray(gate_W[0, 0:D], np.float16), (P, 1))
    BREP = np.full((P, 1), float(np.asarray(gate_b).reshape(-1)[0]), np.float32)
    IDENT = np.eye(P, dtype=np.float16)

    order = np.argsort(edge_dst, kind="stable")
    sd = np.asarray(edge_dst)[order].astype(np.int64)
    ss = np.asarray(edge_src)[order].astype(np.int64)
    bounds = np.searchsorted(sd, np.arange(N_CORES + 1) * NPC)

    cores = []
    for c in range(N_CORES):
        lo, hi = int(bounds[c]), int(bounds[c + 1])
        dl = sd[lo:hi] - c * NPC
        src = ss[lo:hi]

        color, uniq, refcnt = _color_ranges(src, dl, NPC)

        # table row assignment: per range, referenced srcs by refcount desc
        tau = np.full(N_NODES_MAX, -1, np.int64)
        g_all = np.full(N_NODES_MAX, -1, np.int8)
        g_all[uniq] = color[uniq]
        used = np.zeros(NRANGE, np.int64)
        rc_full = np.zeros(N_NODES_MAX, np.int64)
        rc_full[uniq] = refcnt
        for g in range(NRANGE):
            nodes_g = uniq[color[uniq] == g]
            nodes_g = nodes_g[np.argsort(-rc_full[nodes_g], kind="stable")]
            assert nodes_g.size <= RCAP[g], (g, nodes_g.size)
            tau[nodes_g] = RSTART[g] + np.arange(nodes_g.size)
            used[g] = nodes_g.size
        unref = np.where(g_all < 0)[0]
        pos = 0
        for g in range(NRANGE):
            free = RCAP[g] - used[g]
            take = min(free, unref.size - pos)
            if take > 0:
                tau[unref[pos:pos + take]] = RSTART[g] + used[g] + np.arange(take)
                used[g] += take
                pos += take
        assert pos == unref.size

        # per-edge range + rank within (dst, range)
        ge = g_all[src].astype(np.int64)
        key = dl * NRANGE + ge
        c_g = np.bincount(key, minlength=NPC * NRANGE).reshape(NPC, NRANGE)
        deg = c_g.sum(axis=1)
        kstart = np.zeros(NPC * NRANGE, np.int64)
        kstart[1:] = np.cumsum(np.bincount(key, minlength=NPC * NRANGE))[:-1]
        ord2 = np.argsort(key, kind="stable")
        trank = np.empty(src.size, np.int64)
        trank[ord2] = np.arange(src.size) - kstart[key[ord2]]

        # window packing: group dsts by worst per-range count
        perm = np.argsort(-(c_g[:, :3].max(axis=1) * 64 + deg), kind="stable")
        rankof = np.empty(NPC, np.int64)
        rankof[perm] = np.arange(NPC)

        ntwg_c = c_g[perm].reshape(NW, P, NRANGE).max(axis=1)  # [NW, 4]
        cores.append(dict(
            dl=dl, src=src, tau=tau, ge=ge, trank=trank, perm=perm,
            rankof=rankof, ntwg=ntwg_c,
        ))

    NTWG = np.maximum.reduce([cc["ntwg"] for cc in cores])     # [NW, 4]
    # every PSUM bank needs >=1 matmul so its start=True zeroing fires
    for w in range(0, NW, 8):
        NTWG[w, 0] = max(NTWG[w, 0], 1)
    key = tuple(int(x) for x in NTWG.reshape(-1))
    icol_off, TOTI = _idx_layout(NTWG)

    in_maps = []
    for c in range(N_CORES):
        cc = cores[c]
        tau, ge, trank, perm, rankof = (cc["tau"], cc["ge"], cc["trank"],
                                        cc["perm"], cc["rankof"])
        dl, src = cc["dl"], cc["src"]

        r = rankof[dl]
        p_arr = r % P
        w_arr = r // P

        # idx stream: per (w,g) block, one int16 per slot at
        # [sel%16, off+sel//16], replicated x8 across partition groups
        idxw = np.zeros((16, TOTI), np.int16)
        for w in range(NW):
            for g in range(NRANGE):
                nt = int(NTWG[w, g])
                if nt == 0:
                    continue
                o = int(icol_off[w, g])
                idxw[:, o:o + nt * 8] = ZROWR[g]
        sel_i = trank * P + p_arr
        col16 = sel_i // 16
        row16 = sel_i % 16
        val = (tau[src] - np.asarray(RSTART, np.int64)[ge]).astype(np.int64)
        dest_col = icol_off[w_arr, ge] + col16
        idxw[row16, dest_col] = val.astype(np.int16)
        IDXW = np.tile(idxw, (8, 1))

        # node id for each table row
        node_of_row = np.full(R2, -1, np.int64)
        node_of_row[tau[tau >= 0]] = np.where(tau >= 0)[0]
        m = node_of_row >= 0
        rows_n = np.where(m, node_of_row, 0)

        # int8 table per range: [q 0:64 | scale*d_src f16 64:66 | gs 66:68]
        tabs = {}
        hdense = np.zeros((P, R2 // P, D), np.float16)
        for g in range(NRANGE):
            nr = RCAP[g] + 1
            rs = slice(RSTART[g], RSTART[g] + nr)
            tab = np.zeros((nr, EL), np.int8)
            mg = m[rs]
            tab[mg, 0:D] = q_all[rows_n[rs][mg]]
            scp = np.zeros(nr, np.float16)
            scp[mg] = (scale[rows_n[rs][mg]]
                       * d_pad[rows_n[rs][mg]]).astype(np.float16)
            scp[ZROWR[g]] = 0.0
            tab[ZROWR[g], :] = 0
            tab[:, D:D + 2] = scp.view(np.int8).reshape(nr, 2)
            tabs[f"tab{g}"] = tab
        # dense f16 rows for on-device gs compute: hdense[p, j] = h16[row j*128+p]
        hd = np.zeros((R2, D), np.float16)
        hd[m] = h16_all[node_of_row[m]]
        hdense = np.ascontiguousarray(
            hd.reshape(R2 // P, P, D).transpose(1, 0, 2))

        # local dst features (window-ordered) for gd; d_dst column
        nodes_loc = perm + c * NPC
        hloc = np.ascontiguousarray(
            h16_all[nodes_loc].reshape(NW, P, D).transpose(1, 0, 2))
        dcol = np.ascontiguousarray(
            d_pad[nodes_loc].reshape(NW, P).T).astype(np.float32)

        in_maps.append({
            **tabs, "hdense": hdense, "hloc": hloc, "dcol": dcol,
            "wsrc": WSRC, "wdst": WDST, "brep": BREP, "ident": IDENT,
            "idxw": np.ascontiguousarray(IDXW),
            "_perm": perm,
        })
    return in_maps, key


def _raw_gather(eng, mybir, out_ap, in_ap, idxs_ap, num_idxs, elem_bytes):
    """dma_gather with elem_size < 256B (content-only fetch); row stride
    encoded in 256B units. Semantics verified on hardware (v3) and in the
    interpreter (micro tests)."""
    _in_ap = eng.lower_ap_dma(in_ap, for_custom_bir_dma=True)
    _idxs_ap = eng.lower_ap(idxs_ap)
    _out_ap = eng.lower_ap(out_ap)
    return eng.add_instruction(
        mybir.InstDMAGatherAnt(
            name=eng.bass.get_next_instruction_name(),
            ins=[*_in_ap, _idxs_ap,
                 eng.lower_val_access(eng.to_reg(num_idxs))],
            outs=[_out_ap],
            transpose=False,
            num_idxs=num_idxs,
            elem_size=elem_bytes,
            stride_bytes_256=EL // 256,
            gen_mode=0,
            single_packet=False,
            queue_num=0,
            sbuf_tokens_per_rank=0,
            sbuf_free_dim_per_rank=0,
            sbuf_free_dim_pad_per_rank=0,
            sbuf_byte_offset=0,
        ))


def _build_program(key):
    import concourse.bacc as bacc
    import concourse.tile as tile
    from concourse import bass, mybir

    NTWG = np.asarray(key, np.int64).reshape(NW, NRANGE)
    icol_off, TOTI = _idx_layout(NTWG)
    first_mm, last_mm = _mm_stream(NTWG)
    S2MAX = int(max(sum(int(NTWG[w, g]) for w in ws)
                    for ws in GRPS for g in range(NRANGE)))
    S2MAX = max(S2MAX, 1)

    f32, f16 = mybir.dt.float32, mybir.dt.float16
    i16, i8 = mybir.dt.int16, mybir.dt.int8

    nc = bacc.Bacc("TRN2", target_bir_lowering=False, debug=False,
                   num_devices=N_CORES)
    PE = nc.engines[mybir.EngineType.PE]
    tab_d = [nc.dram_tensor(f"tab{g}", [RCAP[g] + 1, EL], i8,
                            kind="ExternalInput") for g in range(NRANGE)]
    hdense_d = nc.dram_tensor("hdense", [P, R2 // P, D], f16,
                              kind="ExternalInput")
    hloc_d = nc.dram_tensor("hloc", [P, NW, D], f16, kind="ExternalInput")
    dcol_d = nc.dram_tensor("dcol", [P, NW], f32, kind="ExternalInput")
    wsrc_d = nc.dram_tensor("wsrc", [P, D], f16, kind="ExternalInput")
    wdst_d = nc.dram_tensor("wdst", [P, D], f16, kind="ExternalInput")
    brep_d = nc.dram_tensor("brep", [P, 1], f32, kind="ExternalInput")
    ident_d = nc.dram_tensor("ident", [P, P], f16, kind="ExternalInput")
    idxw_d = nc.dram_tensor("idxw", [P, TOTI], i16, kind="ExternalInput")
    z_d = nc.dram_tensor("z", [P, NW, D], f16, kind="ExternalOutput")

    with tile.TileContext(nc) as tc:
        with tc.tile_pool(name="const", bufs=1) as cp, \
             tc.tile_pool(name="gsb", bufs=2) as gp, \
             tc.tile_pool(name="mainb", bufs=2) as mpb, \
             tc.tile_pool(name="mains", bufs=3) as mps, \
             tc.psum_pool(name="pp", bufs=2) as pp:
            dcol_t = cp.tile([P, NW], f32)
            nc.sync.dma_start(out=dcol_t[:], in_=dcol_d[:, :])
            wsrc_t = cp.tile([P, D], f16)
            nc.sync.dma_start(out=wsrc_t[:], in_=wsrc_d[:, :])
            wdst_t = cp.tile([P, D], f16)
            nc.sync.dma_start(out=wdst_t[:], in_=wdst_d[:, :])
            brep_t = cp.tile([P, 1], f32)
            nc.sync.dma_start(out=brep_t[:], in_=brep_d[:, :])
            ident_t = cp.tile([P, P], f16)
            nc.sync.dma_start(out=ident_t[:], in_=ident_d[:, :])

            # ---- gd for local (window-permuted) dst nodes ----
            hloc_t = cp.tile([P, NW, D], f16)
            nc.sync.dma_start(out=hloc_t[:], in_=hloc_d[:, :, :])
            nc.vector.tensor_tensor(
                out=hloc_t[:], in0=hloc_t[:],
                in1=wdst_t[:].rearrange("p (a e) -> p a e",
                                        a=1).to_broadcast([P, NW, D]),
                op=mybir.AluOpType.mult)
            width = D
            while width > 2:
                half = width // 2
                nc.vector.tensor_tensor(
                    out=hloc_t[:, :, 0:half], in0=hloc_t[:, :, 0:half],
                    in1=hloc_t[:, :, half:width], op=mybir.AluOpType.add)
                width = half
            gdw_t = cp.tile([P, NW], f32)
            nc.vector.tensor_tensor(
                out=gdw_t[:], in0=hloc_t[:, :, 0],
                in1=hloc_t[:, :, 1], op=mybir.AluOpType.add)
            nc.vector.tensor_scalar(
                out=gdw_t[:], in0=gdw_t[:], scalar1=brep_t[:, 0:1],
                scalar2=None, op0=mybir.AluOpType.add)

            # ---- gs per range: dense f16 rows -> h @ W_src -> table col ----
            def emit_gs_range(g):
                nj = (RCAP[g] + 1) // P          # j-columns in this range
                j0 = RSTART[g] // P
                gs_g = gp.tile([P, 256], f16, tag="gsg")
                for s in range(0, nj, GS_CHUNK):
                    w_ = min(GS_CHUNK, nj - s)
                    hd8 = gp.tile([P, GS_CHUNK, D], f16, tag="hd8")
                    nc.sync.dma_start(out=hd8[:, 0:w_, :],
                                      in_=hdense_d[:, j0 + s:j0 + s + w_, :])
                    nc.vector.tensor_tensor(
                        out=hd8[:, 0:w_, :], in0=hd8[:, 0:w_, :],
                        in1=wsrc_t[:].rearrange("p (a e) -> p a e",
                                                a=1).to_broadcast([P, w_, D]),
                        op=mybir.AluOpType.mult)
                    width = D
                    while width > 1:
                        half = width // 2
                        nc.vector.tensor_tensor(
                            out=hd8[:, 0:w_, 0:half], in0=hd8[:, 0:w_, 0:half],
                            in1=hd8[:, 0:w_, half:width],
                            op=mybir.AluOpType.add)
                        width = half
                    nc.vector.tensor_copy(out=gs_g[:, s:s + w_],
                                          in_=hd8[:, 0:w_, 0])
                view = tab_d[g][:, 66:68].bitcast(f16).rearrange(
                    "(j p) one -> p j one", p=P)
                nc.sync.dma_start(out=view, in_=gs_g[:, 0:nj].rearrange(
                    "p (j one) -> p j one", one=1))

            emit_gs_range(0)

            # ---- main loop: sections x ranges x groups ----
            pending_gs = [g for g in (1, 2, 3)
                          if int(NTWG[:, g].sum()) > 0]

            for si, sec in enumerate(SECS):
                psec = pp.tile([P, 32, D], f32, tag="ps")
                for g in range(NRANGE):
                    # overlap later-range gs builds with earlier passes
                    if pending_gs:
                        emit_gs_range(pending_gs.pop(0))
                    for ws in sec:
                        S2 = sum(int(NTWG[w, g]) for w in ws)
                        if S2 == 0:
                            continue
                        toff = {}
                        t_ = 0
                        for w in ws:
                            toff[w] = t_
                            t_ += int(NTWG[w, g])
                        blk = int(icol_off[ws[0], g])

                        ga = mpb.tile([P, S2MAX, CB], i8, tag="ga")
                        th = mps.tile([P, S2MAX], f16, tag="th")
                        th2 = mps.tile([P, S2MAX, 2], f16, tag="th2")
                        msgf = mpb.tile([P, S2MAX, D], f16, tag="msgf")
                        idx_t = mps.tile([P, S2MAX * 8], i16, tag="idx")
                        nc.sync.dma_start(
                            out=idx_t[:, 0:S2 * 8],
                            in_=idxw_d[:, blk:blk + S2 * 8])
                        for c0 in range(0, S2, 64):
                            cn = min(64, S2 - c0)
                            _raw_gather(nc.gpsimd, mybir,
                                        ga[:, c0:c0 + cn, :],
                                        tab_d[g][:, 0:CB],
                                        idx_t[:, c0 * 8:(c0 + cn) * 8],
                                        cn * P, CB)
                        gs_ap = ga[:, 0:S2, 66:68].bitcast(f16)
                        sc_ap = ga[:, 0:S2, 64:66].bitcast(f16)
                        for w in ws:
                            nt = int(NTWG[w, g])
                            if nt == 0:
                                continue
                            o = toff[w]
                            nc.vector.tensor_scalar(
                                out=th[:, o:o + nt],
                                in0=gs_ap[:, o:o + nt, 0],
                                scalar1=gdw_t[:, w:w + 1], scalar2=None,
                                op0=mybir.AluOpType.add)
                        nc.scalar.activation(
                            out=th[:, 0:S2], in_=th[:, 0:S2],
                            func=mybir.ActivationFunctionType.Tanh)
                        nc.vector.scalar_tensor_tensor(
                            out=th2[:, 0:S2, :],
                            in0=th[:, 0:S2].rearrange(
                                "p (s a) -> p s a", a=1).to_broadcast([P, S2, 2]),
                            scalar=1.0,
                            in1=sc_ap.to_broadcast([P, S2, 2]),
                            op0=mybir.AluOpType.mult, op1=mybir.AluOpType.mult)
                        nc.scalar.activation(
                            out=msgf[:, 0:S2, :], in_=ga[:, 0:S2, 0:D],
                            func=mybir.ActivationFunctionType.Copy)
                        nc.vector.tensor_tensor(
                            out=msgf[:, 0:S2, :].rearrange(
                                "p s (c two) -> p s c two", two=2),
                            in0=msgf[:, 0:S2, :].rearrange(
                                "p s (c two) -> p s c two", two=2),
                            in1=th2[:, 0:S2, :].rearrange(
                                "p s (a two) -> p s a two", a=1
                            ).to_broadcast([P, S2, D // 2, 2]),
                            op=mybir.AluOpType.mult)
                        for w in ws:
                            nt = int(NTWG[w, g])
                            if nt == 0:
                                continue
                            o = toff[w]
                            b = w // 8
                            for t in range(nt):
                                PE.matmul(
                                    out=psec[:, w - SW0[si], :],
                                    lhsT=ident_t[:],
                                    rhs=msgf[:, o + t, :],
                                    start=(first_mm[b] == (g, w, t)),
                                    stop=(last_mm[b] == (g, w, t)))
                # evacuate this section's PSUM with d_dst fused
                zo = mps.tile([P, 32, D], f16, tag="zo")
                for w in range(SW0[si], SW0[si] + SNW[si]):
                    nc.scalar.activation(
                        out=zo[:, w - SW0[si], :],
                        in_=psec[:, w - SW0[si], :],
                        func=mybir.ActivationFunctionType.Copy,
                        scale=dcol_t[:, w:w + 1])
                nc.sync.dma_start(
                    out=z_d[:, SW0[si]:SW0[si] + SNW[si], :],
                    in_=zo[:, 0:SNW[si], :])

    nc.compile()
    return nc


_CACHE = {}


def kernel(h, d, gate_W, gate_b, edge_src, edge_dst):
    from concourse.bass_utils import run_bass_kernel_spmd

    N = h.shape[0]
    in_maps, key = _host_prep(h, d, gate_W, gate_b, edge_src, edge_dst)
    if key not in _CACHE:
        _CACHE[key] = _build_program(key)
    nc = _CACHE[key]
    perms = [m.pop("_perm") for m in in_maps]
    res = run_bass_kernel_spmd(nc, in_maps, core_ids=list(range(N_CORES)))
    z = np.empty((N_CORES * NPC, D), np.float32)
    for c in range(N_CORES):
        zc = res.results[c]["z"].astype(np.float32)    # [128, NW, 64]
        zperm = zc.transpose(1, 0, 2).reshape(NPC, D)  # rank-major
        z[perms[c] + c * NPC] = zperm
    return np.ascontiguousarray(z[:N]).astype(np.float32)
